# revision 14
# baseline (speedup 1.0000x reference)
"""Multi-head attention forward on 8 Trainium2 NeuronCores.

Sharding: core c = 2*b + g handles batch b (of 4) and head-group g (8 of 16
heads). Each core computes its group's attention output projected through its
slice of w_proj (row-parallel); the host sums the two partial products per
batch and adds the bias terms.

Math notes (exact identities, not approximations):
  - the key bias b_k adds a per-query constant to every score row, which
    softmax ignores;
  - the value bias b_v passes through attention unchanged (attn rows sum to 1)
    so its projection b_v @ w_proj is folded into the host-side bias;
  - the 1/sqrt(64) score scale is folded into w_q / b_q (exact: power of two).

The softmax exp stream on the scalar engine (ACT, 1 elem/cycle/lane @1.2GHz,
33.5M exps/core = ~290us) is the hard floor; the kernel is structured so that
stream starts as early as possible and never waits:
  - x / w_qkv / w_v travel in fp16 (halves input DMA; qkv-projection matmuls
    run fp16 x fp16, fp32 PSUM accumulation);
  - the qkv projection for head-pair 0 runs first; attention starts right
    after while the remaining projections and the V pass fill tensor-engine
    slack (Tile's readiness-driven scheduler pulls them into ACT-gated gaps);
  - scores are computed transposed (ST = kT.T @ qT, [s_j, s_i]) so the exp
    output PT (fp16) feeds the PV matmul directly as the moving operand;
  - V carries an appended ones column, so PV row 64 is the softmax
    denominator Z; normalization happens on the evacuated SBUF copy via
    vector.reciprocal_approx_fast (51 ULP, ~5x faster than the iterative
    DVE reciprocal) + GPSIMD partition-broadcast, off the ACT path;
  - head pairs (2t, 2t+1) sit in partition halves {0..63}/{64..127} of shared
    qk tiles: their K=64 score matmuls target different PE row groups with
    identical moving free-addresses and can execute concurrently;
  - the output projection for sequence chunk c streams right after head-pair
    3 finishes chunk c, so the projection + output DMA hide inside the tail
    of the exp stream.
"""

import numpy as np

import concourse.bass as bass
import concourse.tile as tile
from concourse import bacc, mybir
from concourse import bass_utils

F32 = mybir.dt.float32
F32R = mybir.dt.float32r
F16 = mybir.dt.float16
AF = mybir.ActivationFunctionType

B, S, D = 4, 2048, 1024
H, HD = 16, 64
HG = 8            # heads per core (group)
N_CORES = 8
KT = D // 128     # 8 k-tiles over the embedding dim
ST16 = S // 128   # 16 tiles over sequence

_CACHE = {}


def _build():
    nc = bacc.Bacc("TRN2", target_bir_lowering=False, debug=False,
                   num_devices=N_CORES)
    xt_d = nc.dram_tensor("xt", [D, S], F16, kind="ExternalInput").ap()
    wqk_d = nc.dram_tensor("wqk", [D, 2 * HG * HD], F16, kind="ExternalInput").ap()
    wv_d = nc.dram_tensor("wv", [D, HG * HD], F16, kind="ExternalInput").ap()
    wp_d = nc.dram_tensor("wp", [HG * HD, D], F16, kind="ExternalInput").ap()
    bq_d = nc.dram_tensor("bq", [128, 4], F32, kind="ExternalInput").ap()
    out_d = nc.dram_tensor("out", [S, D], F32, kind="ExternalOutput").ap()

    with tile.TileContext(nc) as tc:
        with (
            tc.tile_pool(name="persist", bufs=1) as pp,
            tc.tile_pool(name="stp", bufs=1, space="PSUM") as stp,
            tc.tile_pool(name="pop", bufs=1, space="PSUM") as pop,
            tc.tile_pool(name="ap_", bufs=1, space="PSUM") as ap_,
            tc.tile_pool(name="ptp", bufs=1) as ptp,
            tc.tile_pool(name="np_", bufs=2) as np_,
            tc.tile_pool(name="bcp", bufs=2) as bcp,
            tc.tile_pool(name="yp", bufs=2) as yp,
        ):
            # ---- persistent SBUF tensors ----
            qk_sb = [pp.tile([128, S], F16, name=f"qk{m}", tag=f"qk{m}")
                     for m in range(8)]
            # v stays f32r: the fp16 weight path interleaves stationary
            # column pairs (even output rows read the wrong columns), so the
            # 65-wide [v | ones] PV stationary must use the fp32r path.
            # Per-head stride 68 (272B, 16B-aligned); col 64 = ones (PV row 64
            # = softmax denominator Z), cols 65-67 pad (never read).
            v_sb = [pp.tile([128, HG, 68], F32R, name=f"v{j}", tag=f"v{j}")
                    for j in range(ST16)]
            at_sb = [pp.tile([128, S], F16, name=f"at{t}", tag=f"at{t}")
                     for t in range(4)]
            wp_sb = [pp.tile([128, D], F16, name=f"wp{t}", tag=f"wp{t}")
                     for t in range(4)]
            xt_sb = [pp.tile([128, S], F16, name=f"xt{k}", tag=f"xt{k}")
                     for k in range(KT)]
            wqk_sb = [pp.tile([128, 1024], F16, name=f"wqk{k}",
                              tag=f"wqk{k}") for k in range(KT)]
            wv_sb = [pp.tile([128, 512], F16, name=f"wv{k}", tag=f"wv{k}")
                     for k in range(KT)]
            bq_sb = pp.tile([128, 4], F32, tag="bq")
            oz_sb = pp.tile([128, HG, 1], F32, tag="oz")

            # ---- input DMAs, in priority order: the qkv projection for
            # head-pair 0 (wq0 + xt) gates the start of the exp stream.
            nc.sync.dma_start(bq_sb[:], bq_d)
            for k in range(KT):
                nc.sync.dma_start(xt_sb[k][:], xt_d[k * 128:(k + 1) * 128, :])
                nc.sync.dma_start(wv_sb[k][:], wv_d[k * 128:(k + 1) * 128, :])
            for k in range(KT):
                nc.sync.dma_start(wqk_sb[k][:], wqk_d[k * 128:(k + 1) * 128, :])
            for t in range(4):
                nc.sync.dma_start(wp_sb[t][:], wp_d[t * 128:(t + 1) * 128, :])

            nc.vector.memset(oz_sb[:], 1.0)
            wa = pp.tile([128, 128], F32, tag="wa")
            wb = pp.tile([128, 512], F32, tag="wb")
            nc.vector.memset(wa[:], 1.0)
            nc.vector.memset(wb[:], 1.0)

            def emit_qk_pair(mp, chains=((0, 0), (1, 0), (1, 1), (1, 2),
                                         (1, 3), (0, 1), (0, 2), (0, 3))):
                """qkv projection for head-pair mp: fills qk_sb[mp] (q, scaled,
                +bias) and qk_sb[4+mp] (k). 2-bank accumulation chains; the
                first score group needs q chunk 0 + the k chunks (j order),
                so emit those chains first."""
                for half, n in chains:
                    if True:
                        m = mp if half == 0 else 4 + mp
                        pa = ap_.tile([128, 512], F32, tag="a", bufs=2,
                                      name=f"pa{m}{n}")
                        for k in range(KT):
                            nc.tensor.matmul(
                                pa[:],
                                wqk_sb[k][:, half * 512 + mp * 128:
                                          half * 512 + (mp + 1) * 128],
                                xt_sb[k][:, n * 512:(n + 1) * 512],
                                start=(k == 0), stop=(k == KT - 1))
                        dst = qk_sb[m][:, n * 512:(n + 1) * 512]
                        if half == 0:
                            nc.vector.tensor_scalar_add(dst, pa[:],
                                                        bq_sb[:, mp:mp + 1])
                        else:
                            nc.vector.tensor_copy(dst, pa[:])

            def emit_v(si0, si1):
                """V = x @ wv, natural layout, + ones column. One accumulation
                chain per sequence tile."""
                for si in range(si0, si1):
                    pb = ap_.tile([128, 512], F32, tag="a", bufs=2,
                                  name=f"pb{si}")
                    for k in range(KT):
                        nc.tensor.matmul(
                            pb[:],
                            xt_sb[k][:, si * 128:(si + 1) * 128],
                            wv_sb[k][:],
                            start=(k == 0), stop=(k == KT - 1))
                    nc.vector.tensor_copy(
                        v_sb[si][:, :, 0:HD],
                        pb[:].rearrange("p (h d) -> p h d", h=HG))
                    nc.vector.tensor_copy(v_sb[si][:, :, HD:HD + 1],
                                          oz_sb[:])

            def emit_attn_group(t, c):
                """Attention for head pair (2t, 2t+1), query chunk c (512)."""
                qT = qk_sb[t]
                kT = qk_sb[4 + t]
                po = [pop.tile([HD + 1, 512], F32, tag="po", bufs=2,
                               name=f"po{t}{c}{hh}") for hh in range(2)]
                for j in range(ST16):
                    st = stp.tile([128, 1024], F32, tag="st", bufs=2,
                                  name=f"st{t}{c}{j}")
                    for hh in range(2):
                        nc.tensor.matmul(
                            st[:, hh * 512:(hh + 1) * 512],
                            kT[hh * 64:(hh + 1) * 64, j * 128:(j + 1) * 128],
                            qT[hh * 64:(hh + 1) * 64, c * 512:(c + 1) * 512],
                            start=True, stop=True)
                    pt = ptp.tile([128, 1024], F32R, tag="pt", bufs=4,
                                  name=f"pt{t}{c}{j}")
                    nc.scalar.activation(pt[:], st[:], AF.Exp,
                                         bias=0.0, scale=1.0)
                    for hh in range(2):
                        nc.tensor.matmul(
                            po[hh][:],
                            v_sb[j][:, 2 * t + hh, 0:HD + 1],
                            pt[:, hh * 512:(hh + 1) * 512],
                            start=(j == 0), stop=(j == ST16 - 1))
                # evacuate po (unnormalized) + Z row, then normalize in SBUF
                za = np_.tile([1, 1024], F32, tag="za")
                slots = []
                for hh in range(2):
                    sl = at_sb[t][hh * 64:hh * 64 + 64,
                                  c * 512:(c + 1) * 512]
                    slots.append(sl)
                    nc.vector.tensor_copy(sl, po[hh][0:64, :])
                    nc.vector.tensor_copy(
                        za[:, hh * 512:(hh + 1) * 512], po[hh][64:65, :])
                inv = np_.tile([1, 1024], F32, tag="zb")
                nc.vector.reciprocal_approx_fast(inv[:], za[:])
                bc = bcp.tile([128, 1024], F32, tag="bc")
                nc.gpsimd.partition_broadcast(bc[:], inv[:])
                for hh in range(2):
                    nc.vector.tensor_mul(
                        slots[hh], slots[hh],
                        bc[hh * 64:hh * 64 + 64, hh * 512:(hh + 1) * 512])

            def emit_proj_chunk(c):
                """out[:, c-chunk] = attnT.T @ wp for the 4 sequence tiles of
                query chunk c. at_sb for all 4 pairs must be final."""
                for s4 in range(4):
                    si = c * 4 + s4
                    y = yp.tile([128, 1024], F32, tag="y")
                    for nch in range(2):
                        py = ap_.tile([128, 512], F32, tag="a", bufs=2,
                                      name=f"py{si}{nch}")
                        for tt in range(4):
                            nc.tensor.matmul(
                                py[:],
                                at_sb[tt][:, si * 128:(si + 1) * 128],
                                wp_sb[tt][:, nch * 512:(nch + 1) * 512],
                                start=(tt == 0), stop=(tt == 3))
                        nc.vector.tensor_copy(
                            y[:, nch * 512:(nch + 1) * 512], py[:])
                    nc.sync.dma_start(
                        out_d[si * 128:(si + 1) * 128, :], y[:])

            # ---- emission order = scheduler priority ----
            # Emission = dependency order (Tile gives sequential-program
            # semantics: producers MUST be emitted before consumers). Filler
            # work is wrapped in a low-priority band (negative high_priority
            # offset) so the scheduler always prefers the ACT-feeding
            # attention stream and pulls fillers into its slack.
            LOW = -1000000
            emit_qk_pair(0, chains=((0, 0), (1, 0), (1, 1), (1, 2), (1, 3)))
            emit_v(0, 8)           # gates PV j=0..7 of the first group
            with tc.high_priority(offset=LOW):
                # PE clock warmup (HAM un-throttle needs sustained matmul
                # activity while the input DMAs land). Idle "st" slots.
                for _ in range(16):
                    wu = stp.tile([128, 512], F32, tag="st", bufs=2,
                                  name="warm")
                    nc.tensor.matmul(wu[:], wa[:], wb[:], start=True,
                                     stop=True)
                # ACT exp table warm-load off the critical path
                warm_pt = pp.tile([1, 128], F32, tag="warmpt")
                nc.scalar.activation(warm_pt[:], wa[0:1, 0:128], AF.Exp,
                                     bias=0.0, scale=1.0)
                emit_v(8, 16)
                emit_qk_pair(0, chains=((0, 1), (0, 2), (0, 3)))
            emit_attn_group(0, 0)
            emit_attn_group(0, 1)
            with tc.high_priority(offset=LOW):
                emit_qk_pair(1)
            emit_attn_group(0, 2)
            emit_attn_group(0, 3)
            emit_attn_group(1, 0)
            emit_attn_group(1, 1)
            with tc.high_priority(offset=LOW):
                emit_qk_pair(2)
            emit_attn_group(1, 2)
            emit_attn_group(1, 3)
            emit_attn_group(2, 0)
            emit_attn_group(2, 1)
            with tc.high_priority(offset=LOW):
                emit_qk_pair(3)
            emit_attn_group(2, 2)
            emit_attn_group(2, 3)
            for c in range(4):
                emit_attn_group(3, c)
                with tc.high_priority(offset=LOW):
                    emit_proj_chunk(c)  # projection fills pair-3 slack

    nc.compile()
    return nc


def _prep_inputs(x, w_qkv, b_qkv, w_proj):
    """Host-side shard prep: slice per head-group, fold scale, transpose x."""
    in_maps = []
    xt_b = [np.ascontiguousarray(x[b].T.astype(np.float16)) for b in range(B)]
    for c in range(N_CORES):
        b, g = c // 2, c % 2
        cs = g * 512
        wq = (w_qkv[:, cs:cs + 512] * 0.125).astype(np.float16)
        wk = w_qkv[:, 1024 + cs:1024 + cs + 512].astype(np.float16)
        wv = w_qkv[:, 2048 + cs:2048 + cs + 512].astype(np.float16)
        bq = (b_qkv[cs:cs + 512] * 0.125).reshape(4, 128).T
        in_maps.append({
            "xt": xt_b[b],
            "wqk": np.ascontiguousarray(np.concatenate([wq, wk], axis=1)),
            "wv": np.ascontiguousarray(wv),
            "wp": np.ascontiguousarray(
                w_proj[g * 512:(g + 1) * 512, :].astype(np.float16)),
            "bq": np.ascontiguousarray(bq.astype(np.float32)),
        })
    return in_maps


def kernel(x, w_qkv, b_qkv, w_proj, b_proj, _trace=False):
    x = np.asarray(x, np.float32)
    w_qkv = np.asarray(w_qkv, np.float32)
    b_qkv = np.asarray(b_qkv, np.float32)
    w_proj = np.asarray(w_proj, np.float32)
    b_proj = np.asarray(b_proj, np.float32)

    if "nc" not in _CACHE:
        _CACHE["nc"] = _build()
    nc = _CACHE["nc"]

    in_maps = _prep_inputs(x, w_qkv, b_qkv, w_proj)
    res = bass_utils.run_bass_kernel_spmd(
        nc, in_maps, core_ids=list(range(N_CORES)), trace=_trace)

    # host-side bias: b_proj plus the value-bias path through w_proj
    bias = b_proj + b_qkv[2048:3072].astype(np.float64) @ w_proj.astype(np.float64)
    bias = bias.astype(np.float32)
    out = np.empty((B, S, D), np.float32)
    for b in range(B):
        out[b] = res.results[2 * b]["out"] + res.results[2 * b + 1]["out"] + bias
    if _trace:
        return out, res
    return out


# revision 18
# speedup vs baseline: 1.0002x; 1.0002x over previous
"""Multi-head attention forward on 8 Trainium2 NeuronCores.

Sharding: core c = 2*b + g handles batch b (of 4) and head-group g (8 of 16
heads). Each core computes its group's attention output projected through its
slice of w_proj (row-parallel); the host sums the two partial products per
batch and adds the bias terms.

Math notes (exact identities, not approximations):
  - the key bias b_k adds a per-query constant to every score row, which
    softmax ignores;
  - the value bias b_v passes through attention unchanged (attn rows sum to 1)
    so its projection b_v @ w_proj is folded into the host-side bias;
  - the 1/sqrt(64) score scale is folded into w_q / b_q (exact: power of two).

The softmax exp stream on the scalar engine (ACT, 1 elem/cycle/lane @1.2GHz,
33.5M exps/core = ~283us) is the hard floor; the kernel is structured so that
stream starts early and rarely waits (measured ~408us vs 502us for the
phase-sequential baseline):
  - x / w_qkv / w_v / w_proj travel in fp16 (smaller input DMA; projection and
    score matmuls run fp16 x fp16 with fp32 PSUM accumulation);
  - emission order follows dependency order (Tile gives sequential-program
    value semantics - consumers emitted before producers read uninitialized
    memory), while filler work (PE warmup, the V pass, later qkv pairs, the
    output projection) is wrapped in a NEGATIVE tc.high_priority offset so
    the scheduler always prefers the ACT-feeding attention stream and pulls
    fillers into its slack;
  - scores are computed transposed (ST = kT.T @ qT, [s_j, s_i]) so the exp
    output PT (f32r) feeds the PV matmul directly as the moving operand;
  - V (f32r, 16B-aligned head stride) carries an appended ones column, so PV
    row 64 is the softmax denominator Z; normalization happens on the
    evacuated SBUF copy via vector.reciprocal_approx_fast (51 ULP, ~5x faster
    than the iterative DVE reciprocal) + GPSIMD partition-broadcast, off the
    ACT path;
  - head pairs (2t, 2t+1) sit in partition halves {0..63}/{64..127} of shared
    qk tiles: their K=64 score matmuls target different PE row groups with
    identical moving free-addresses and execute concurrently (measured
    ~4ns start deltas);
  - the output projection for sequence chunk c streams right after head-pair
    3 finishes chunk c, so most of the projection + output DMA hides inside
    the tail of the exp stream.
"""

import numpy as np

import concourse.bass as bass
import concourse.tile as tile
from concourse import bacc, mybir
from concourse import bass_utils

F32 = mybir.dt.float32
F32R = mybir.dt.float32r
F16 = mybir.dt.float16
AF = mybir.ActivationFunctionType

B, S, D = 4, 2048, 1024
H, HD = 16, 64
HG = 8            # heads per core (group)
N_CORES = 8
KT = D // 128     # 8 k-tiles over the embedding dim
ST16 = S // 128   # 16 tiles over sequence

_CACHE = {}


def _build():
    nc = bacc.Bacc("TRN2", target_bir_lowering=False, debug=False,
                   num_devices=N_CORES)
    xt_d = nc.dram_tensor("xt", [D, S], F16, kind="ExternalInput").ap()
    wqk_d = nc.dram_tensor("wqk", [D, 2 * HG * HD], F16, kind="ExternalInput").ap()
    wv_d = nc.dram_tensor("wv", [D, HG * HD], F16, kind="ExternalInput").ap()
    wp_d = nc.dram_tensor("wp", [HG * HD, D], F16, kind="ExternalInput").ap()
    bq_d = nc.dram_tensor("bq", [128, 4], F32, kind="ExternalInput").ap()
    out_d = nc.dram_tensor("out", [S, D], F32, kind="ExternalOutput").ap()

    with tile.TileContext(nc) as tc:
        with (
            tc.tile_pool(name="persist", bufs=1) as pp,
            tc.tile_pool(name="stp", bufs=1, space="PSUM") as stp,
            tc.tile_pool(name="pop", bufs=1, space="PSUM") as pop,
            tc.tile_pool(name="ap_", bufs=1, space="PSUM") as ap_,
            tc.tile_pool(name="ptp", bufs=1) as ptp,
            tc.tile_pool(name="np_", bufs=2) as np_,
            tc.tile_pool(name="bcp", bufs=2) as bcp,
            tc.tile_pool(name="yp", bufs=2) as yp,
        ):
            # ---- persistent SBUF tensors ----
            qk_sb = [pp.tile([128, S], F16, name=f"qk{m}", tag=f"qk{m}")
                     for m in range(8)]
            # v stays f32r: the fp16 weight path interleaves stationary
            # column pairs (even output rows read the wrong columns), so the
            # 65-wide [v | ones] PV stationary must use the fp32r path.
            # Per-head stride 68 (272B, 16B-aligned); col 64 = ones (PV row 64
            # = softmax denominator Z), cols 65-67 pad (never read).
            v_sb = [pp.tile([128, HG, 68], F32R, name=f"v{j}", tag=f"v{j}")
                    for j in range(ST16)]
            at_sb = [pp.tile([128, S], F16, name=f"at{t}", tag=f"at{t}")
                     for t in range(4)]
            wp_sb = [pp.tile([128, D], F16, name=f"wp{t}", tag=f"wp{t}")
                     for t in range(4)]
            xt_sb = [pp.tile([128, S], F16, name=f"xt{k}", tag=f"xt{k}")
                     for k in range(KT)]
            wqk_sb = [pp.tile([128, 1024], F16, name=f"wqk{k}",
                              tag=f"wqk{k}") for k in range(KT)]
            wv_sb = [pp.tile([128, 512], F16, name=f"wv{k}", tag=f"wv{k}")
                     for k in range(KT)]
            bq_sb = pp.tile([128, 4], F32, tag="bq")
            oz_sb = pp.tile([128, HG, 1], F32, tag="oz")

            # ---- input DMAs, in priority order: the qkv projection for
            # head-pair 0 (wq0 + xt) gates the start of the exp stream.
            nc.sync.dma_start(bq_sb[:], bq_d)
            for k in range(KT):
                nc.sync.dma_start(xt_sb[k][:], xt_d[k * 128:(k + 1) * 128, :])
                nc.sync.dma_start(wv_sb[k][:], wv_d[k * 128:(k + 1) * 128, :])
            for k in range(KT):
                nc.sync.dma_start(wqk_sb[k][:], wqk_d[k * 128:(k + 1) * 128, :])
            for t in range(4):
                nc.sync.dma_start(wp_sb[t][:], wp_d[t * 128:(t + 1) * 128, :])

            nc.vector.memset(oz_sb[:], 1.0)
            wa = pp.tile([128, 128], F32, tag="wa")
            wb = pp.tile([128, 512], F32, tag="wb")
            nc.vector.memset(wa[:], 1.0)
            nc.vector.memset(wb[:], 1.0)

            def emit_qk_pair(mp, chains=((0, 0), (1, 0), (1, 1), (1, 2),
                                         (1, 3), (0, 1), (0, 2), (0, 3))):
                """qkv projection for head-pair mp: fills qk_sb[mp] (q, scaled,
                +bias) and qk_sb[4+mp] (k). 2-bank accumulation chains; the
                first score group needs q chunk 0 + the k chunks (j order),
                so emit those chains first."""
                for half, n in chains:
                    if True:
                        m = mp if half == 0 else 4 + mp
                        pa = ap_.tile([128, 512], F32, tag="a", bufs=2,
                                      name=f"pa{m}{n}")
                        for k in range(KT):
                            nc.tensor.matmul(
                                pa[:],
                                wqk_sb[k][:, half * 512 + mp * 128:
                                          half * 512 + (mp + 1) * 128],
                                xt_sb[k][:, n * 512:(n + 1) * 512],
                                start=(k == 0), stop=(k == KT - 1))
                        dst = qk_sb[m][:, n * 512:(n + 1) * 512]
                        if half == 0:
                            nc.vector.tensor_scalar_add(dst, pa[:],
                                                        bq_sb[:, mp:mp + 1])
                        else:
                            nc.vector.tensor_copy(dst, pa[:])

            def emit_v(si0, si1):
                """V = x @ wv, natural layout, + ones column. One accumulation
                chain per sequence tile."""
                for si in range(si0, si1):
                    pb = ap_.tile([128, 512], F32, tag="a", bufs=2,
                                  name=f"pb{si}")
                    for k in range(KT):
                        nc.tensor.matmul(
                            pb[:],
                            xt_sb[k][:, si * 128:(si + 1) * 128],
                            wv_sb[k][:],
                            start=(k == 0), stop=(k == KT - 1))
                    nc.vector.tensor_copy(
                        v_sb[si][:, :, 0:HD],
                        pb[:].rearrange("p (h d) -> p h d", h=HG))
                    nc.vector.tensor_copy(v_sb[si][:, :, HD:HD + 1],
                                          oz_sb[:])

            def emit_attn_group(t, c):
                """Attention for head pair (2t, 2t+1), query chunk c (512)."""
                qT = qk_sb[t]
                kT = qk_sb[4 + t]
                po = [pop.tile([HD + 1, 512], F32, tag="po", bufs=2,
                               name=f"po{t}{c}{hh}") for hh in range(2)]
                for j in range(ST16):
                    st = stp.tile([128, 1024], F32, tag="st", bufs=2,
                                  name=f"st{t}{c}{j}")
                    for hh in range(2):
                        nc.tensor.matmul(
                            st[:, hh * 512:(hh + 1) * 512],
                            kT[hh * 64:(hh + 1) * 64, j * 128:(j + 1) * 128],
                            qT[hh * 64:(hh + 1) * 64, c * 512:(c + 1) * 512],
                            start=True, stop=True)
                    pt = ptp.tile([128, 1024], F32R, tag="pt", bufs=4,
                                  name=f"pt{t}{c}{j}")
                    nc.scalar.activation(pt[:], st[:], AF.Exp,
                                         bias=0.0, scale=1.0)
                    for hh in range(2):
                        nc.tensor.matmul(
                            po[hh][:],
                            v_sb[j][:, 2 * t + hh, 0:HD + 1],
                            pt[:, hh * 512:(hh + 1) * 512],
                            start=(j == 0), stop=(j == ST16 - 1))
                # evacuate po (unnormalized) + Z row, then normalize in SBUF
                za = np_.tile([1, 1024], F32, tag="za")
                slots = []
                for hh in range(2):
                    sl = at_sb[t][hh * 64:hh * 64 + 64,
                                  c * 512:(c + 1) * 512]
                    slots.append(sl)
                    nc.vector.tensor_copy(sl, po[hh][0:64, :])
                    nc.vector.tensor_copy(
                        za[:, hh * 512:(hh + 1) * 512], po[hh][64:65, :])
                inv = np_.tile([1, 1024], F32, tag="zb")
                nc.vector.reciprocal_approx_fast(inv[:], za[:])
                bc = bcp.tile([128, 1024], F32, tag="bc")
                nc.gpsimd.partition_broadcast(bc[:], inv[:])
                for hh in range(2):
                    nc.vector.tensor_mul(
                        slots[hh], slots[hh],
                        bc[hh * 64:hh * 64 + 64, hh * 512:(hh + 1) * 512])

            def emit_proj_chunk(c):
                """out[:, c-chunk] = attnT.T @ wp for the 4 sequence tiles of
                query chunk c. at_sb for all 4 pairs must be final."""
                for s4 in range(4):
                    si = c * 4 + s4
                    y = yp.tile([128, 1024], F32, tag="y")
                    for nch in range(2):
                        py = ap_.tile([128, 512], F32, tag="a", bufs=2,
                                      name=f"py{si}{nch}")
                        for tt in range(4):
                            nc.tensor.matmul(
                                py[:],
                                at_sb[tt][:, si * 128:(si + 1) * 128],
                                wp_sb[tt][:, nch * 512:(nch + 1) * 512],
                                start=(tt == 0), stop=(tt == 3))
                        nc.vector.tensor_copy(
                            y[:, nch * 512:(nch + 1) * 512], py[:])
                    nc.sync.dma_start(
                        out_d[si * 128:(si + 1) * 128, :], y[:])

            # ---- emission order = scheduler priority ----
            # Emission = dependency order (Tile gives sequential-program
            # semantics: producers MUST be emitted before consumers). Filler
            # work is wrapped in a low-priority band (negative high_priority
            # offset) so the scheduler always prefers the ACT-feeding
            # attention stream and pulls fillers into its slack.
            LOW = -1000000
            emit_qk_pair(0)
            emit_v(0, 8)           # gates PV j=0..7 of the first group
            with tc.high_priority(offset=LOW):
                # PE clock warmup (HAM un-throttle needs sustained matmul
                # activity while the input DMAs land). Idle "st" slots.
                for _ in range(16):
                    wu = stp.tile([128, 512], F32, tag="st", bufs=2,
                                  name="warm")
                    nc.tensor.matmul(wu[:], wa[:], wb[:], start=True,
                                     stop=True)
                # ACT exp table warm-load off the critical path
                warm_pt = pp.tile([1, 128], F32, tag="warmpt")
                nc.scalar.activation(warm_pt[:], wa[0:1, 0:128], AF.Exp,
                                     bias=0.0, scale=1.0)
                emit_v(8, 16)
            emit_attn_group(0, 0)
            emit_attn_group(0, 1)
            with tc.high_priority(offset=LOW):
                emit_qk_pair(1)
            emit_attn_group(0, 2)
            emit_attn_group(0, 3)
            emit_attn_group(1, 0)
            emit_attn_group(1, 1)
            with tc.high_priority(offset=LOW):
                emit_qk_pair(2)
            emit_attn_group(1, 2)
            emit_attn_group(1, 3)
            emit_attn_group(2, 0)
            emit_attn_group(2, 1)
            with tc.high_priority(offset=LOW):
                emit_qk_pair(3)
            emit_attn_group(2, 2)
            emit_attn_group(2, 3)
            for c in range(4):
                emit_attn_group(3, c)
                with tc.high_priority(offset=LOW):
                    emit_proj_chunk(c)  # projection fills pair-3 slack

    nc.compile()
    return nc


def _prep_inputs(x, w_qkv, b_qkv, w_proj):
    """Host-side shard prep: slice per head-group, fold scale, transpose x."""
    in_maps = []
    xt_b = [np.ascontiguousarray(x[b].T.astype(np.float16)) for b in range(B)]
    for c in range(N_CORES):
        b, g = c // 2, c % 2
        cs = g * 512
        wq = (w_qkv[:, cs:cs + 512] * 0.125).astype(np.float16)
        wk = w_qkv[:, 1024 + cs:1024 + cs + 512].astype(np.float16)
        wv = w_qkv[:, 2048 + cs:2048 + cs + 512].astype(np.float16)
        bq = (b_qkv[cs:cs + 512] * 0.125).reshape(4, 128).T
        in_maps.append({
            "xt": xt_b[b],
            "wqk": np.ascontiguousarray(np.concatenate([wq, wk], axis=1)),
            "wv": np.ascontiguousarray(wv),
            "wp": np.ascontiguousarray(
                w_proj[g * 512:(g + 1) * 512, :].astype(np.float16)),
            "bq": np.ascontiguousarray(bq.astype(np.float32)),
        })
    return in_maps


def kernel(x, w_qkv, b_qkv, w_proj, b_proj, _trace=False):
    x = np.asarray(x, np.float32)
    w_qkv = np.asarray(w_qkv, np.float32)
    b_qkv = np.asarray(b_qkv, np.float32)
    w_proj = np.asarray(w_proj, np.float32)
    b_proj = np.asarray(b_proj, np.float32)

    if "nc" not in _CACHE:
        _CACHE["nc"] = _build()
    nc = _CACHE["nc"]

    in_maps = _prep_inputs(x, w_qkv, b_qkv, w_proj)
    res = bass_utils.run_bass_kernel_spmd(
        nc, in_maps, core_ids=list(range(N_CORES)), trace=_trace)

    # host-side bias: b_proj plus the value-bias path through w_proj
    bias = b_proj + b_qkv[2048:3072].astype(np.float64) @ w_proj.astype(np.float64)
    bias = bias.astype(np.float32)
    out = np.empty((B, S, D), np.float32)
    for b in range(B):
        out[b] = res.results[2 * b]["out"] + res.results[2 * b + 1]["out"] + bias
    if _trace:
        return out, res
    return out


# revision 19
# speedup vs baseline: 1.0054x; 1.0051x over previous
"""Multi-head attention forward on 8 Trainium2 NeuronCores.

Sharding: core c = 2*b + g handles batch b (of 4) and head-group g (8 of 16
heads). Each core computes its group's attention output projected through its
slice of w_proj (row-parallel); the host sums the two partial products per
batch and adds the bias terms.

Math notes (exact identities, not approximations):
  - the key bias b_k adds a per-query constant to every score row, which
    softmax ignores;
  - the value bias b_v passes through attention unchanged (attn rows sum to 1)
    so its projection b_v @ w_proj is folded into the host-side bias;
  - the 1/sqrt(64) score scale is folded into w_q / b_q (exact: power of two).

The softmax exp stream on the scalar engine (ACT, 1 elem/cycle/lane @1.2GHz,
33.5M exps/core = ~283us) is the hard floor; the kernel is structured so that
stream starts early and rarely waits (measured ~408us vs 502us for the
phase-sequential baseline):
  - x / w_qkv / w_v / w_proj travel in fp16 (smaller input DMA; projection and
    score matmuls run fp16 x fp16 with fp32 PSUM accumulation);
  - emission order follows dependency order (Tile gives sequential-program
    value semantics - consumers emitted before producers read uninitialized
    memory), while filler work (PE warmup, the V pass, later qkv pairs, the
    output projection) is wrapped in a NEGATIVE tc.high_priority offset so
    the scheduler always prefers the ACT-feeding attention stream and pulls
    fillers into its slack;
  - scores are computed transposed (ST = kT.T @ qT, [s_j, s_i]) so the exp
    output PT (f32r) feeds the PV matmul directly as the moving operand;
  - V (f32r, 16B-aligned head stride) carries an appended ones column, so PV
    row 64 is the softmax denominator Z; normalization happens on the
    evacuated SBUF copy via vector.reciprocal_approx_fast (51 ULP, ~5x faster
    than the iterative DVE reciprocal) + GPSIMD partition-broadcast, off the
    ACT path;
  - head pairs (2t, 2t+1) sit in partition halves {0..63}/{64..127} of shared
    qk tiles: their K=64 score matmuls target different PE row groups with
    identical moving free-addresses and execute concurrently (measured
    ~4ns start deltas);
  - the output projection for sequence chunk c streams right after head-pair
    3 finishes chunk c, so most of the projection + output DMA hides inside
    the tail of the exp stream.
"""

import numpy as np

import concourse.bass as bass
import concourse.tile as tile
from concourse import bacc, mybir
from concourse import bass_utils

F32 = mybir.dt.float32
F32R = mybir.dt.float32r
F16 = mybir.dt.float16
AF = mybir.ActivationFunctionType

B, S, D = 4, 2048, 1024
H, HD = 16, 64
HG = 8            # heads per core (group)
N_CORES = 8
KT = D // 128     # 8 k-tiles over the embedding dim
ST16 = S // 128   # 16 tiles over sequence

_CACHE = {}


def _build():
    nc = bacc.Bacc("TRN2", target_bir_lowering=False, debug=False,
                   num_devices=N_CORES)
    xt_d = nc.dram_tensor("xt", [D, S], F16, kind="ExternalInput").ap()
    wqk_d = nc.dram_tensor("wqk", [D, 2 * HG * HD], F16, kind="ExternalInput").ap()
    wv_d = nc.dram_tensor("wv", [D, HG * HD], F16, kind="ExternalInput").ap()
    wp_d = nc.dram_tensor("wp", [HG * HD, D], F16, kind="ExternalInput").ap()
    bq_d = nc.dram_tensor("bq", [128, 4], F32, kind="ExternalInput").ap()
    out_d = nc.dram_tensor("out", [S, D], F32, kind="ExternalOutput").ap()

    with tile.TileContext(nc) as tc:
        with (
            tc.tile_pool(name="persist", bufs=1) as pp,
            tc.tile_pool(name="stp", bufs=1, space="PSUM") as stp,
            tc.tile_pool(name="pop", bufs=1, space="PSUM") as pop,
            tc.tile_pool(name="ap_", bufs=1, space="PSUM") as ap_,
            tc.tile_pool(name="ptp", bufs=1) as ptp,
            tc.tile_pool(name="np_", bufs=2) as np_,
            tc.tile_pool(name="bcp", bufs=1) as bcp,
            tc.tile_pool(name="yp", bufs=2) as yp,
        ):
            # ---- persistent SBUF tensors ----
            qk_sb = [pp.tile([128, S], F16, name=f"qk{m}", tag=f"qk{m}")
                     for m in range(8)]
            # v stays f32r: the fp16 weight path interleaves stationary
            # column pairs (even output rows read the wrong columns), so the
            # 65-wide [v | ones] PV stationary must use the fp32r path.
            # Per-head stride 68 (272B, 16B-aligned); col 64 = ones (PV row 64
            # = softmax denominator Z), cols 65-67 pad (never read).
            v_sb = [pp.tile([128, HG, 68], F32R, name=f"v{j}", tag=f"v{j}")
                    for j in range(ST16)]
            at_sb = [pp.tile([128, S], F16, name=f"at{t}", tag=f"at{t}")
                     for t in range(4)]
            wp_sb = [pp.tile([128, D], F16, name=f"wp{t}", tag=f"wp{t}")
                     for t in range(4)]
            xt_sb = [pp.tile([128, S], F16, name=f"xt{k}", tag=f"xt{k}")
                     for k in range(KT)]
            wqk_sb = [pp.tile([128, 1024], F16, name=f"wqk{k}",
                              tag=f"wqk{k}") for k in range(KT)]
            wv_sb = [pp.tile([128, 512], F16, name=f"wv{k}", tag=f"wv{k}")
                     for k in range(KT)]
            bq_sb = pp.tile([128, 4], F32, tag="bq")
            oz_sb = pp.tile([128, HG, 1], F32, tag="oz")

            # ---- input DMAs, in priority order: the qkv projection for
            # head-pair 0 (wq0 + xt) gates the start of the exp stream.
            nc.sync.dma_start(bq_sb[:], bq_d)
            for k in range(KT):
                nc.sync.dma_start(xt_sb[k][:], xt_d[k * 128:(k + 1) * 128, :])
                nc.sync.dma_start(wv_sb[k][:], wv_d[k * 128:(k + 1) * 128, :])
            for k in range(KT):
                nc.sync.dma_start(wqk_sb[k][:], wqk_d[k * 128:(k + 1) * 128, :])
            for t in range(4):
                nc.sync.dma_start(wp_sb[t][:], wp_d[t * 128:(t + 1) * 128, :])

            nc.vector.memset(oz_sb[:], 1.0)
            wa = pp.tile([128, 128], F32, tag="wa")
            wb = pp.tile([128, 512], F32, tag="wb")
            nc.vector.memset(wa[:], 1.0)
            nc.vector.memset(wb[:], 1.0)

            def emit_qk_pair(mp, chains=((0, 0), (1, 0), (1, 1), (1, 2),
                                         (1, 3), (0, 1), (0, 2), (0, 3))):
                """qkv projection for head-pair mp: fills qk_sb[mp] (q, scaled,
                +bias) and qk_sb[4+mp] (k). 2-bank accumulation chains; the
                first score group needs q chunk 0 + the k chunks (j order),
                so emit those chains first."""
                for half, n in chains:
                    if True:
                        m = mp if half == 0 else 4 + mp
                        pa = ap_.tile([128, 512], F32, tag="a", bufs=2,
                                      name=f"pa{m}{n}")
                        for k in range(KT):
                            nc.tensor.matmul(
                                pa[:],
                                wqk_sb[k][:, half * 512 + mp * 128:
                                          half * 512 + (mp + 1) * 128],
                                xt_sb[k][:, n * 512:(n + 1) * 512],
                                start=(k == 0), stop=(k == KT - 1))
                        dst = qk_sb[m][:, n * 512:(n + 1) * 512]
                        if half == 0:
                            nc.vector.tensor_scalar_add(dst, pa[:],
                                                        bq_sb[:, mp:mp + 1])
                        else:
                            nc.vector.tensor_copy(dst, pa[:])

            def emit_v(si0, si1):
                """V = x @ wv, natural layout, + ones column. One accumulation
                chain per sequence tile."""
                for si in range(si0, si1):
                    pb = ap_.tile([128, 512], F32, tag="a", bufs=2,
                                  name=f"pb{si}")
                    for k in range(KT):
                        nc.tensor.matmul(
                            pb[:],
                            xt_sb[k][:, si * 128:(si + 1) * 128],
                            wv_sb[k][:],
                            start=(k == 0), stop=(k == KT - 1))
                    nc.vector.tensor_copy(
                        v_sb[si][:, :, 0:HD],
                        pb[:].rearrange("p (h d) -> p h d", h=HG))
                    nc.vector.tensor_copy(v_sb[si][:, :, HD:HD + 1],
                                          oz_sb[:])

            def emit_attn_group(t, c):
                """Attention for head pair (2t, 2t+1), query chunk c (512)."""
                qT = qk_sb[t]
                kT = qk_sb[4 + t]
                po = [pop.tile([HD + 1, 512], F32, tag="po", bufs=2,
                               name=f"po{t}{c}{hh}") for hh in range(2)]
                for j in range(ST16):
                    st = stp.tile([128, 1024], F32, tag="st", bufs=2,
                                  name=f"st{t}{c}{j}")
                    for hh in range(2):
                        nc.tensor.matmul(
                            st[:, hh * 512:(hh + 1) * 512],
                            kT[hh * 64:(hh + 1) * 64, j * 128:(j + 1) * 128],
                            qT[hh * 64:(hh + 1) * 64, c * 512:(c + 1) * 512],
                            start=True, stop=True)
                    pt = ptp.tile([128, 1024], F32R, tag="pt", bufs=6,
                                  name=f"pt{t}{c}{j}")
                    nc.scalar.activation(pt[:], st[:], AF.Exp,
                                         bias=0.0, scale=1.0)
                    for hh in range(2):
                        nc.tensor.matmul(
                            po[hh][:],
                            v_sb[j][:, 2 * t + hh, 0:HD + 1],
                            pt[:, hh * 512:(hh + 1) * 512],
                            start=(j == 0), stop=(j == ST16 - 1))
                # evacuate po (unnormalized) + Z row, then normalize in SBUF
                za = np_.tile([1, 1024], F32, tag="za")
                slots = []
                for hh in range(2):
                    sl = at_sb[t][hh * 64:hh * 64 + 64,
                                  c * 512:(c + 1) * 512]
                    slots.append(sl)
                    nc.vector.tensor_copy(sl, po[hh][0:64, :])
                    nc.vector.tensor_copy(
                        za[:, hh * 512:(hh + 1) * 512], po[hh][64:65, :])
                inv = np_.tile([1, 1024], F32, tag="zb")
                nc.vector.reciprocal_approx_fast(inv[:], za[:])
                bc = bcp.tile([128, 1024], F32, tag="bc")
                nc.gpsimd.partition_broadcast(bc[:], inv[:])
                for hh in range(2):
                    nc.vector.tensor_mul(
                        slots[hh], slots[hh],
                        bc[hh * 64:hh * 64 + 64, hh * 512:(hh + 1) * 512])

            def emit_proj_chunk(c):
                """out[:, c-chunk] = attnT.T @ wp for the 4 sequence tiles of
                query chunk c. at_sb for all 4 pairs must be final."""
                for s4 in range(4):
                    si = c * 4 + s4
                    y = yp.tile([128, 1024], F32, tag="y")
                    for nch in range(2):
                        py = ap_.tile([128, 512], F32, tag="a", bufs=2,
                                      name=f"py{si}{nch}")
                        for tt in range(4):
                            nc.tensor.matmul(
                                py[:],
                                at_sb[tt][:, si * 128:(si + 1) * 128],
                                wp_sb[tt][:, nch * 512:(nch + 1) * 512],
                                start=(tt == 0), stop=(tt == 3))
                        nc.vector.tensor_copy(
                            y[:, nch * 512:(nch + 1) * 512], py[:])
                    nc.sync.dma_start(
                        out_d[si * 128:(si + 1) * 128, :], y[:])

            # ---- emission order = scheduler priority ----
            # Emission = dependency order (Tile gives sequential-program
            # semantics: producers MUST be emitted before consumers). Filler
            # work is wrapped in a low-priority band (negative high_priority
            # offset) so the scheduler always prefers the ACT-feeding
            # attention stream and pulls fillers into its slack.
            LOW = -1000000
            emit_qk_pair(0)
            emit_v(0, 8)           # gates PV j=0..7 of the first group
            with tc.high_priority(offset=LOW):
                # PE clock warmup (HAM un-throttle needs sustained matmul
                # activity while the input DMAs land). Idle "st" slots.
                for _ in range(16):
                    wu = stp.tile([128, 512], F32, tag="st", bufs=2,
                                  name="warm")
                    nc.tensor.matmul(wu[:], wa[:], wb[:], start=True,
                                     stop=True)
                # ACT exp table warm-load off the critical path
                warm_pt = pp.tile([1, 128], F32, tag="warmpt")
                nc.scalar.activation(warm_pt[:], wa[0:1, 0:128], AF.Exp,
                                     bias=0.0, scale=1.0)
                emit_v(8, 16)
            emit_attn_group(0, 0)
            emit_attn_group(0, 1)
            with tc.high_priority(offset=LOW):
                emit_qk_pair(1)
            emit_attn_group(0, 2)
            emit_attn_group(0, 3)
            emit_attn_group(1, 0)
            emit_attn_group(1, 1)
            with tc.high_priority(offset=LOW):
                emit_qk_pair(2)
            emit_attn_group(1, 2)
            emit_attn_group(1, 3)
            emit_attn_group(2, 0)
            emit_attn_group(2, 1)
            with tc.high_priority(offset=LOW):
                emit_qk_pair(3)
            emit_attn_group(2, 2)
            emit_attn_group(2, 3)
            for c in range(4):
                emit_attn_group(3, c)
                with tc.high_priority(offset=LOW):
                    emit_proj_chunk(c)  # projection fills pair-3 slack

    nc.compile()
    return nc


def _prep_inputs(x, w_qkv, b_qkv, w_proj):
    """Host-side shard prep: slice per head-group, fold scale, transpose x."""
    in_maps = []
    xt_b = [np.ascontiguousarray(x[b].T.astype(np.float16)) for b in range(B)]
    for c in range(N_CORES):
        b, g = c // 2, c % 2
        cs = g * 512
        wq = (w_qkv[:, cs:cs + 512] * 0.125).astype(np.float16)
        wk = w_qkv[:, 1024 + cs:1024 + cs + 512].astype(np.float16)
        wv = w_qkv[:, 2048 + cs:2048 + cs + 512].astype(np.float16)
        bq = (b_qkv[cs:cs + 512] * 0.125).reshape(4, 128).T
        in_maps.append({
            "xt": xt_b[b],
            "wqk": np.ascontiguousarray(np.concatenate([wq, wk], axis=1)),
            "wv": np.ascontiguousarray(wv),
            "wp": np.ascontiguousarray(
                w_proj[g * 512:(g + 1) * 512, :].astype(np.float16)),
            "bq": np.ascontiguousarray(bq.astype(np.float32)),
        })
    return in_maps


def kernel(x, w_qkv, b_qkv, w_proj, b_proj, _trace=False):
    x = np.asarray(x, np.float32)
    w_qkv = np.asarray(w_qkv, np.float32)
    b_qkv = np.asarray(b_qkv, np.float32)
    w_proj = np.asarray(w_proj, np.float32)
    b_proj = np.asarray(b_proj, np.float32)

    if "nc" not in _CACHE:
        _CACHE["nc"] = _build()
    nc = _CACHE["nc"]

    in_maps = _prep_inputs(x, w_qkv, b_qkv, w_proj)
    res = bass_utils.run_bass_kernel_spmd(
        nc, in_maps, core_ids=list(range(N_CORES)), trace=_trace)

    # host-side bias: b_proj plus the value-bias path through w_proj
    bias = b_proj + b_qkv[2048:3072].astype(np.float64) @ w_proj.astype(np.float64)
    bias = bias.astype(np.float32)
    out = np.empty((B, S, D), np.float32)
    for b in range(B):
        out[b] = res.results[2 * b]["out"] + res.results[2 * b + 1]["out"] + bias
    if _trace:
        return out, res
    return out
